# revision 1
# baseline (speedup 1.0000x reference)
"""Trainium2 Bass kernel for single-head causal attention.

Problem: x[4096,2048]; q/k/v = x@W + b; scores = causal(q k^T / sqrt(d_head));
out = softmax(scores) @ v @ W_O + b_O.

Strategy (8 NeuronCores, SPMD, no collectives):
  Sequence-parallel over query rows: core c owns rows [512c, 512(c+1)).
  Each core computes the full K^T / V projections (replicated -- forced by the
  causal structure without inter-core communication), its own 512-row Q slice,
  masked full-extent attention, and its 512-row output slice. The host
  concatenates the 8 row-blocks.

  All matmuls are laid out to need exactly one on-device transpose
  (softmax weights -> PV lhsT), done on the PE with an identity matrix.
    qT[d,r]   = W_Qs^T-chain:   lhsT=W_Qs tile, rhs=xq
    kT[d,s]   :                 lhsT=W_K tile,  rhs=xT       (-> DRAM scratch)
    v[s,d]    :                 lhsT=xT tile,   rhs=W_V      (-> DRAM scratch)
    scores    :                 lhsT=qT tile,   rhs=kT block (PSUM f32)
    weights   = exp(scores - 25) * mask   (constant-max softmax; row sums kept)
    attnT[d,r]:                 lhsT=v tile,    rhs=wT
    out       :                 lhsT=attnT tile, rhs=W_O, x (1/rowsum) on copy

  Numerics: bf16 matmul inputs, fp32 PSUM accumulation. 1/sqrt(d_head) folded
  into W_Q on host. b_K is a softmax no-op; b_V/b_O folded on host; b_Q is
  zero per the problem spec (asserted).
"""

import math
import os
import sys

for _p in ("/opt/trn_rl_repo",):
    if _p not in sys.path and os.path.isdir(_p):
        sys.path.insert(0, _p)

import numpy as np
import ml_dtypes

import concourse.bass as bass
import concourse.mybir as mybir
import concourse.tile as tile
from concourse import bass_utils
from concourse.masks import make_identity
from contextlib import ExitStack

P = 128
NB = 512  # matmul moving free dim / PSUM bank
BF16 = mybir.dt.bfloat16
F32 = mybir.dt.float32
AF = mybir.ActivationFunctionType
EXP_SHIFT = -25.0  # constant-max softmax shift; |scores| << 25 for this data

LAST_RESULT = None  # test.py reads exec_time_ns from here


def split_multi_waits(nc):
    """This neuronxcc walrus lowers at most ONE sync wait per instruction
    (setupSyncWait: 'Too many sync wait commands'). Tile emits multi-wait
    instructions; hoist all but the last wait onto preceding EventSemaphore
    instructions on the same engine (strictly more conservative ordering)."""
    n_split = 0

    def fix(blocks):
        nonlocal n_split
        for b in blocks:
            out = []
            changed = False
            for inst in b.instructions:
                si = inst.sync_info
                waits = list(si.on_wait) if si is not None and si.on_wait else []
                if len(waits) > 1:
                    for j, w in enumerate(waits[:-1]):
                        es = mybir.InstEventSemaphore(
                            name=f"{inst.name}-esw{j}", ins=[], outs=[])
                        es.engine = inst.engine
                        es.sync_info = mybir.SyncInfo(on_wait=[w], on_update=[])
                        out.append(es)
                        n_split += 1
                    inst.sync_info = mybir.SyncInfo(
                        on_wait=[waits[-1]],
                        on_update=list(si.on_update) if si.on_update else [])
                    changed = True
                out.append(inst)
            if changed:
                b.instructions = out

    for fn in nc.m.functions:
        fix(fn.blocks)
    return n_split


def build_bass(S, D, R, n_cores=8, trace_label=""):
    DT = D // P    # d tiles (16)
    SKT = S // P   # key tiles (32)
    SKB = S // NB  # key blocks (8)
    RQ = R // P    # q row tiles (4)
    DNB = D // NB  # d blocks (4)

    nc = bass.Bass("TRN2", target_bir_lowering=False, debug=False,
                   enable_asserts=False, num_devices=n_cores)

    xq_d = nc.dram_tensor("xq", [D, R], BF16, kind="ExternalInput").ap()
    xT_d = nc.dram_tensor("xT", [D, S], BF16, kind="ExternalInput").ap()
    wq_d = nc.dram_tensor("wq", [D, D], BF16, kind="ExternalInput").ap()
    wk_d = nc.dram_tensor("wk", [D, D], BF16, kind="ExternalInput").ap()
    wv_d = nc.dram_tensor("wv", [D, D], BF16, kind="ExternalInput").ap()
    wo_d = nc.dram_tensor("wo", [D, D], BF16, kind="ExternalInput").ap()
    mask_d = nc.dram_tensor("mask", [R, S], BF16, kind="ExternalInput").ap()
    out_d = nc.dram_tensor("out", [R, D], F32, kind="ExternalOutput").ap()
    kts_d = nc.dram_tensor("kts", [D, S], BF16, kind="Internal").ap()
    vs_d = nc.dram_tensor("vs", [S, D], BF16, kind="Internal").ap()

    def col3(ap_2d, j0, w):
        # DRAM [A, B] column slice [:, j0:j0+w] -> SBUF layout [P, A//P, w]
        return ap_2d[:, j0:j0 + w].rearrange("(o p) n -> p o n", p=P)

    with ExitStack() as ctx:
        tc = ctx.enter_context(tile.TileContext(nc))
        ps_mm = ctx.enter_context(tc.tile_pool(name="ps_mm", bufs=6, space="PSUM"))
        ps_tr = ctx.enter_context(tc.tile_pool(name="ps_tr", bufs=2, space="PSUM"))
        persist = ctx.enter_context(tc.tile_pool(name="persist", bufs=1))
        stage = ctx.enter_context(tc.tile_pool(name="stage", bufs=4))

        qT = persist.tile([P, DT, R], BF16, tag="qT")
        ident = persist.tile([P, P], BF16, tag="ident")
        make_identity(nc, ident)
        expb = persist.tile([P, 1], F32, tag="expb")
        nc.vector.memset(expb, EXP_SHIFT)

        # ---------------- phase 1: qT = (W_Q*scale)^T chain on xq ----------
        with tc.tile_pool(name="p1", bufs=3) as p1, \
             tc.tile_pool(name="p1s", bufs=1) as p1s:
            xq = p1s.tile([P, DT, R], BF16, tag="xq")
            nc.sync.dma_start(xq, xq_d.rearrange("(o p) n -> p o n", p=P))
            for m in range(DT):
                wqc = p1.tile([P, DT, P], BF16, tag="wcol")
                nc.sync.dma_start(wqc, col3(wq_d, m * P, P))
                for r in range(R // NB):
                    ps = ps_mm.tile([P, NB], F32, tag="mm")
                    for k in range(DT):
                        nc.tensor.matmul(ps, wqc[:, k, :], xq[:, k, r * NB:(r + 1) * NB],
                                         start=(k == 0), stop=(k == DT - 1))
                    nc.scalar.activation(qT[:, m, r * NB:(r + 1) * NB], ps, AF.Copy)

        # ---------------- phase 2+3: kT and v projections -> DRAM ----------
        with tc.tile_pool(name="early", bufs=1) as early, \
             tc.tile_pool(name="p2", bufs=2) as p2:
            xT = early.tile([P, DT, S], BF16, tag="xT")
            for nb in range(SKB):
                nc.sync.dma_start(xT[:, :, nb * NB:(nb + 1) * NB], col3(xT_d, nb * NB, NB))
            # kT[m-block, n-block] = sum_k W_K[k,m]^T @ xT[k,n]
            for m in range(DT):
                wkc = p2.tile([P, DT, P], BF16, tag="wcol")
                nc.sync.dma_start(wkc, col3(wk_d, m * P, P))
                for nb in range(SKB):
                    ps = ps_mm.tile([P, NB], F32, tag="mm")
                    for k in range(DT):
                        nc.tensor.matmul(ps, wkc[:, k, :], xT[:, k, nb * NB:(nb + 1) * NB],
                                         start=(k == 0), stop=(k == DT - 1))
                    st = stage.tile([P, NB], BF16, tag="stg")
                    nc.scalar.activation(st, ps, AF.Copy)
                    nc.sync.dma_start(kts_d[m * P:(m + 1) * P, nb * NB:(nb + 1) * NB], st)
            # v[m-block(keys), nb-block(d)] = sum_k xT[k, mkeys]^T @ W_V[k, nb]
            for nb in range(DNB):
                wvb = p2.tile([P, DT, NB], BF16, tag="wblk")
                nc.sync.dma_start(wvb, col3(wv_d, nb * NB, NB))
                for m in range(SKT):
                    ps = ps_mm.tile([P, NB], F32, tag="mm")
                    for k in range(DT):
                        nc.tensor.matmul(ps, xT[:, k, m * P:(m + 1) * P], wvb[:, k, :],
                                         start=(k == 0), stop=(k == DT - 1))
                    st = stage.tile([P, NB], BF16, tag="stg")
                    nc.vector.tensor_copy(st, ps)
                    nc.sync.dma_start(vs_d[m * P:(m + 1) * P, nb * NB:(nb + 1) * NB], st)

        # ---------------- phase 4: scores -> exp -> mask -> wT -------------
        late = ctx.enter_context(tc.tile_pool(name="late", bufs=1))
        wT = late.tile([P, SKT, R], BF16, tag="wT")
        rsum = persist.tile([P, RQ, SKB], F32, tag="rsum")
        rrec = persist.tile([P, RQ, 1], F32, tag="rrec")
        with tc.tile_pool(name="p4", bufs=4) as p4, \
             tc.tile_pool(name="p4s", bufs=1) as p4s:
            mask = p4s.tile([P, RQ, S], BF16, tag="mask")
            nc.sync.dma_start(mask, mask_d.rearrange("(o p) n -> p o n", p=P))
            for nb in range(SKB):
                ktb = p4.tile([P, DT, NB], BF16, tag="ktb")
                nc.sync.dma_start(ktb, col3(kts_d, nb * NB, NB))
                for mq in range(RQ):
                    ps = ps_mm.tile([P, NB], F32, tag="mm")
                    for k in range(DT):
                        nc.tensor.matmul(ps, qT[:, k, mq * P:(mq + 1) * P], ktb[:, k, :],
                                         start=(k == 0), stop=(k == DT - 1))
                    wgt = p4.tile([P, NB], BF16, tag="wgt")
                    nc.scalar.activation(wgt, ps, AF.Exp, bias=expb)
                    nc.vector.tensor_mul(wgt, wgt, mask[:, mq, nb * NB:(nb + 1) * NB])
                    nc.vector.reduce_sum(rsum[:, mq, nb:nb + 1], wgt,
                                         axis=mybir.AxisListType.X)
                    # transpose 128x128 blocks: wT[key, qrow]
                    for t in range(NB // P):
                        pt = ps_tr.tile([P, P], BF16, tag="tr")
                        nc.tensor.transpose(pt, wgt[:, t * P:(t + 1) * P], ident)
                        nc.vector.tensor_copy(
                            wT[:, nb * (NB // P) + t, mq * P:(mq + 1) * P], pt)
            for mq in range(RQ):
                nc.vector.reduce_sum(rrec[:, mq, :], rsum[:, mq, :],
                                     axis=mybir.AxisListType.X)
                nc.vector.reciprocal(rrec[:, mq, :], rrec[:, mq, :])

        # ---------------- phase 5: attnT = (weights @ v)^T -----------------
        attnT = late.tile([P, DT, R], BF16, tag="attnT")
        with tc.tile_pool(name="p5", bufs=4) as p5:
            for m in range(DT):
                vcol = p5.tile([P, SKT, P], BF16, tag="vcol")
                nc.sync.dma_start(
                    vcol, vs_d[:, m * P:(m + 1) * P].rearrange("(o p) n -> p o n", p=P))
                ps = ps_mm.tile([P, R], F32, tag="mm")
                for kb in range(SKT):
                    nc.tensor.matmul(ps, vcol[:, kb, :], wT[:, kb, :],
                                     start=(kb == 0), stop=(kb == SKT - 1))
                nc.scalar.activation(attnT[:, m, :], ps, AF.Copy)

        # ---------------- phase 6: out = attn @ W_O, scaled by 1/rowsum ----
        with tc.tile_pool(name="p6", bufs=2) as p6, \
             tc.tile_pool(name="p6s", bufs=1) as p6s:
            out_sb = p6s.tile([P, RQ, D], F32, tag="out")
            for nb in range(DNB):
                wob = p6.tile([P, DT, NB], BF16, tag="wblk")
                nc.sync.dma_start(wob, col3(wo_d, nb * NB, NB))
                for mq in range(RQ):
                    ps = ps_mm.tile([P, NB], F32, tag="mm")
                    for k in range(DT):
                        nc.tensor.matmul(ps, attnT[:, k, mq * P:(mq + 1) * P], wob[:, k, :],
                                         start=(k == 0), stop=(k == DT - 1))
                    nc.scalar.activation(out_sb[:, mq, nb * NB:(nb + 1) * NB], ps,
                                         AF.Copy, scale=rrec[:, mq, :])
            nc.sync.dma_start(out_d.rearrange("(o p) n -> p o n", p=P), out_sb)

    split_multi_waits(nc)
    return nc


def kernel(x, W_Q, W_K, W_V, W_O, b_Q, b_K, b_V, b_O, d_head, trace=False):
    global LAST_RESULT
    x = np.asarray(x, np.float32)
    S, D = x.shape
    n_cores = 8
    R = S // n_cores
    dh = float(np.asarray(d_head))
    scale = 1.0 / math.sqrt(dh)
    bq = np.asarray(b_Q, np.float32)
    assert not np.any(bq), "b_Q != 0 not supported by this kernel"

    bf = ml_dtypes.bfloat16
    xT_b = np.ascontiguousarray(x.T).astype(bf)                      # [D, S]
    wq_b = (np.asarray(W_Q, np.float32) * scale).astype(bf)
    wk_b = np.asarray(W_K, np.float32).astype(bf)
    wv_b = np.asarray(W_V, np.float32).astype(bf)
    wo_b = np.asarray(W_O, np.float32).astype(bf)

    cols = np.arange(S, dtype=np.int64)[None, :]
    in_maps = []
    for c in range(n_cores):
        rows = np.arange(c * R, (c + 1) * R, dtype=np.int64)[:, None]
        in_maps.append({
            "xq": np.ascontiguousarray(xT_b[:, c * R:(c + 1) * R]),
            "xT": xT_b,
            "wq": wq_b, "wk": wk_b, "wv": wv_b, "wo": wo_b,
            "mask": (cols <= rows).astype(bf),
        })

    nc = build_bass(S, D, R, n_cores)
    res = bass_utils.run_bass_kernel_spmd(nc, in_maps, core_ids=list(range(n_cores)),
                                          trace=trace)
    LAST_RESULT = res
    out = np.concatenate([r["out"] for r in res.results], axis=0).astype(np.float32)
    # b_K is a softmax no-op; b_V/b_O fold linearly into the output.
    out += (np.asarray(b_V, np.float32) @ np.asarray(W_O, np.float32)
            + np.asarray(b_O, np.float32))[None, :]
    return out

